# revision 13
# baseline (speedup 1.0000x reference)
"""Fused multi-head attention (B=2, T=2048, D=2048, H=16) on 8 trn2 NeuronCores.

Sharding: core c handles batch b=c//4 and heads [4g, 4g+4), g=c%4 (tensor
parallel over heads x data parallel over batch). Each core computes its
4 heads' contribution to out[b] = attn(x[b]) @ Wo^T; the host sums the 4
partials per batch.

Device algorithm (all matmuls fp32r, PSUM fp32):
  P1a  qT = (Wq_s/sqrt(dh)) @ x^T, kT = Wk_s @ x^T       [E=512, T]
  P1b  v  = x @ Wv_s^T                                    [T, E]
  P2   per i-chunk (512 queries), per head:
         S^T[j,i] = kT_h^T-contract : PSUM -> exp (ACT) -> * exp(mask^T)
         ctx^T[e,i] += v_h^T @ P^T  (PSUM, 16 j-tiles)
         l[i]      += 1^T @ P^T     (PSUM)
         ctx^T *= broadcast(1/l)    (outer-product bcast + DVE mul)
  P3   out[t,dd] = sum_e ctx^T[e,t] * WoT[e,dd]  -> DRAM

Inputs are pre-transposed/sharded/scaled on the host so every device matmul
is a natural [K=128-tile] x [N<=512] fp32r op.
"""

import os

import numpy as np

import concourse.bass as bass
import concourse.mybir as mybir
import concourse.tile as tile
from concourse import bacc
from concourse.bass_utils import run_bass_kernel_spmd

F32 = mybir.dt.float32
F32R = mybir.dt.float32r
EXP = mybir.ActivationFunctionType.Exp

B, T, D, H = 2, 2048, 2048, 16
DH = D // H          # 128
E = 512              # features per core (4 heads)
HPC = 4              # heads per core
NT = T // 128        # 16 token tiles
ND = D // 128        # 16 model-dim tiles
NE = E // 128        # 4 e-tiles per core
NI = T // 512        # 4 i-chunks (moving dim)
NJ = NT              # 16 j-tiles

_NC_CACHE = {}

# per-(jt, ic) mask-block class: 0 = fully masked (skip), 1 = unmasked
# (skip the mask multiply), 2 = mixed (apply exp(mask) elementwise)
SKIP, NOMULT, MIXED = 0, 1, 2


def _build(cls_key):
    cls = np.asarray(cls_key, dtype=np.int64).reshape(NJ, NI)
    nc = bacc.Bacc(None, target_bir_lowering=False, debug=False)
    xt = nc.declare_dram_parameter("xt", [D, T], F32R, isOutput=False)
    wq = nc.declare_dram_parameter("wq", [D, E], F32R, isOutput=False)
    wk = nc.declare_dram_parameter("wk", [D, E], F32R, isOutput=False)
    wv = nc.declare_dram_parameter("wv", [D, E], F32R, isOutput=False)
    wo = nc.declare_dram_parameter("wo", [E, D], F32R, isOutput=False)
    em = nc.declare_dram_parameter("em", [T, T], F32, isOutput=False)
    onk = nc.declare_dram_parameter("onk", [128, 1], F32R, isOutput=False)
    onp = nc.declare_dram_parameter("onp", [1, 128], F32R, isOutput=False)
    out = nc.declare_dram_parameter("out", [T, D], F32, isOutput=True)

    with tile.TileContext(nc) as tc:
        # ---- long-lived residents (stack order: ctx outlives qk/v) -----
        pool_ctx = tc.alloc_tile_pool(name="res_ctx", bufs=1)
        ctx = [pool_ctx.tile([128, T], F32R, name=f"ctx{m}") for m in range(NE)]
        pool_qk = tc.alloc_tile_pool(name="res_qk", bufs=1)
        qT = [pool_qk.tile([128, T], F32R, name=f"qT{m}") for m in range(NE)]
        kT = [pool_qk.tile([128, T], F32R, name=f"kT{m}") for m in range(NE)]

        scope_p1a = nc.named_scope("P1a_qk"); scope_p1a.__enter__()
        # ---- P1a: q/k projections --------------------------------------
        p_w = tc.alloc_tile_pool(name="p1w", bufs=1)
        wq_sb = p_w.tile([128, ND, E], F32R)
        wk_sb = p_w.tile([128, ND, E], F32R)
        for dt in range(ND):
            nc.sync.dma_start(out=wq_sb[:, dt, :], in_=wq.ap()[dt * 128:(dt + 1) * 128, :])
            nc.sync.dma_start(out=wk_sb[:, dt, :], in_=wk.ap()[dt * 128:(dt + 1) * 128, :])
        p_x = tc.alloc_tile_pool(name="p1x", bufs=3)
        p_ps1 = tc.alloc_tile_pool(name="p1ps", bufs=8, space="PSUM")
        for nch in range(NI):
            psq, psk = {}, {}
            for m in range(NE):
                ps_q = p_ps1.tile([128, 512], F32, name="ps_q", bufs=4)
                ps_k = p_ps1.tile([128, 512], F32, name="ps_k", bufs=4)
                psq[m], psk[m] = ps_q, ps_k
            for dt in range(ND):
                xtile = p_x.tile([128, 512], F32R, name="xtile")
                nc.sync.dma_start(
                    out=xtile,
                    in_=xt.ap()[dt * 128:(dt + 1) * 128, nch * 512:(nch + 1) * 512])
                st, sp = dt == 0, dt == ND - 1
                for m in range(NE):
                    nc.tensor.matmul(psq[m], wq_sb[:, dt, m * 128:(m + 1) * 128],
                                     xtile, start=st, stop=sp)
                    nc.tensor.matmul(psk[m], wk_sb[:, dt, m * 128:(m + 1) * 128],
                                     xtile, start=st, stop=sp)
            for m in range(NE):
                nc.scalar.copy(qT[m][:, nch * 512:(nch + 1) * 512], psq[m])
                nc.vector.tensor_copy(kT[m][:, nch * 512:(nch + 1) * 512], psk[m])
        p_ps1.release()
        p_x.release()
        p_w.release()
        scope_p1a.__exit__(None, None, None)
        scope_p1b = nc.named_scope("P1b_v"); scope_p1b.__enter__()

        # ---- P1b: v projection -----------------------------------------
        pool_v = tc.alloc_tile_pool(name="res_v", bufs=1)
        v_sb = pool_v.tile([128, NT, E], F32R)
        p_wv = tc.alloc_tile_pool(name="p1bw", bufs=1)
        wv_sb = p_wv.tile([128, ND, E], F32R)
        for dt in range(ND):
            nc.sync.dma_start(out=wv_sb[:, dt, :], in_=wv.ap()[dt * 128:(dt + 1) * 128, :])
        p_xs = tc.alloc_tile_pool(name="p1bx", bufs=4)
        p_ps2 = tc.alloc_tile_pool(name="p1bps", bufs=2, space="PSUM")
        for tt in range(NT):
            ps_v = p_ps2.tile([128, 512], F32, name="ps_v")
            for dt in range(ND):
                xst = p_xs.tile([128, 128], F32R, name="xst")
                nc.sync.dma_start(
                    out=xst,
                    in_=xt.ap()[dt * 128:(dt + 1) * 128, tt * 128:(tt + 1) * 128])
                nc.tensor.matmul(ps_v, xst, wv_sb[:, dt, :],
                                 start=(dt == 0), stop=(dt == ND - 1))
            nc.scalar.copy(v_sb[:, tt, :], ps_v)
        p_ps2.release()
        p_xs.release()
        p_wv.release()
        scope_p1b.__exit__(None, None, None)
        scope_p2 = nc.named_scope("P2_attn"); scope_p2.__enter__()

        # ---- P2: attention ---------------------------------------------
        p_const = tc.alloc_tile_pool(name="p2c", bufs=1)
        ones_k = p_const.tile([128, 1], F32R)
        ones_p = p_const.tile([1, 128], F32R)
        nc.sync.dma_start(out=ones_k, in_=onk.ap())
        nc.sync.dma_start(out=ones_p, in_=onp.ap())

        p_em = tc.alloc_tile_pool(name="p2em", bufs=3)
        p_pt = tc.alloc_tile_pool(name="p2pt", bufs=3)
        p_ptm = tc.alloc_tile_pool(name="p2ptm", bufs=3)
        p_bs = tc.alloc_tile_pool(name="p2bs", bufs=2)
        p_rr = tc.alloc_tile_pool(name="p2rr", bufs=2)
        ps_ctx_pool = tc.alloc_tile_pool(name="p2psc", bufs=2, space="PSUM")
        ps_l_pool = tc.alloc_tile_pool(name="p2psl", bufs=2, space="PSUM")
        ps_s_pool = tc.alloc_tile_pool(name="p2pss", bufs=3, space="PSUM")
        ps_b_pool = tc.alloc_tile_pool(name="p2psb", bufs=1, space="PSUM")

        for ic in range(NI):
            isl = slice(ic * 512, (ic + 1) * 512)
            surv = [jt for jt in range(NJ) if cls[jt, ic] != SKIP]
            assert surv, f"i-chunk {ic}: every key block masked"
            first, last = surv[0], surv[-1]
            for hp in range(HPC // 2):
                heads = (2 * hp, 2 * hp + 1)
                cps, lps = {}, {}
                for h in heads:
                    ps_c = ps_ctx_pool.tile([128, 512], F32, name="ps_c")
                    ps_l = ps_l_pool.tile([1, 512], F32, name="ps_l")
                    cps[h], lps[h] = ps_c, ps_l
                for jt in surv:
                    if cls[jt, ic] == MIXED:
                        emt = p_em.tile([128, 512], F32, name="emt")
                        nc.sync.dma_start(
                            out=emt, in_=em.ap()[jt * 128:(jt + 1) * 128, isl])
                    for h in heads:
                        ps_s = ps_s_pool.tile([128, 512], F32, name="ps_s")
                        nc.tensor.matmul(
                            ps_s, kT[h][:, jt * 128:(jt + 1) * 128],
                            qT[h][:, isl], start=True, stop=True)
                        pt = p_pt.tile([128, 512], F32R, name="pt")
                        nc.scalar.activation(pt, ps_s, EXP)
                        if cls[jt, ic] == MIXED:
                            ptm = p_ptm.tile([128, 512], F32R, name="ptm")
                            nc.vector.tensor_mul(ptm, pt, emt)
                        else:
                            ptm = pt
                        st, sp = jt == first, jt == last
                        nc.tensor.matmul(
                            cps[h], v_sb[:, jt, h * 128:(h + 1) * 128],
                            ptm, start=st, stop=sp)
                        nc.tensor.matmul(lps[h], ones_k, ptm,
                                         start=st, stop=sp)
                for h in heads:
                    rr = p_rr.tile([1, 512], F32R, name="rr")
                    with nc.allow_low_precision(reason="softmax recip f32r"):
                        nc.vector.reciprocal(rr, lps[h])
                    ps_b = ps_b_pool.tile([128, 512], F32, name="ps_b")
                    nc.tensor.matmul(ps_b, ones_p, rr, start=True, stop=True)
                    bsb = p_bs.tile([128, 512], F32, name="bsb")
                    nc.scalar.copy(bsb, ps_b)
                    nc.vector.tensor_mul(ctx[h][:, isl], cps[h], bsb)
        for p in (ps_b_pool, ps_s_pool, ps_l_pool, ps_ctx_pool,
                  p_rr, p_bs, p_ptm, p_pt, p_em, p_const):
            p.release()
        pool_v.release()
        pool_qk.release()
        scope_p2.__exit__(None, None, None)
        scope_p3 = nc.named_scope("P3_out"); scope_p3.__enter__()

        # ---- P3: output projection -------------------------------------
        p_wo = tc.alloc_tile_pool(name="p3w", bufs=1)
        wo_sb = p_wo.tile([128, NE, D], F32R)
        for et in range(NE):
            nc.sync.dma_start(out=wo_sb[:, et, :], in_=wo.ap()[et * 128:(et + 1) * 128, :])
        p_ot = tc.alloc_tile_pool(name="p3o", bufs=3)
        p_ps3 = tc.alloc_tile_pool(name="p3ps", bufs=3, space="PSUM")
        for tt in range(NT):
            tsl = slice(tt * 128, (tt + 1) * 128)
            for nch in range(NI):
                ps_o = p_ps3.tile([128, 512], F32, name="ps_o")
                for et in range(NE):
                    nc.tensor.matmul(
                        ps_o, ctx[et][:, tsl],
                        wo_sb[:, et, nch * 512:(nch + 1) * 512],
                        start=(et == 0), stop=(et == NE - 1))
                ot = p_ot.tile([128, 512], F32, name="ot")
                nc.scalar.copy(ot, ps_o)
                nc.sync.dma_start(
                    out=out.ap()[tsl, nch * 512:(nch + 1) * 512], in_=ot)
        p_ps3.release()
        p_ot.release()
        p_wo.release()
        pool_ctx.release()
        scope_p3.__exit__(None, None, None)

    nc.compile()
    return nc


def _get_nc(cls_key):
    if cls_key not in _NC_CACHE:
        _NC_CACHE[cls_key] = _build(cls_key)
    return _NC_CACHE[cls_key]


def kernel(x, Wq, Wk, Wv, Wo, attn_mask):
    x = np.asarray(x, dtype=np.float32)
    Wq = np.asarray(Wq, dtype=np.float32)
    Wk = np.asarray(Wk, dtype=np.float32)
    Wv = np.asarray(Wv, dtype=np.float32)
    Wo = np.asarray(Wo, dtype=np.float32)
    mask = np.asarray(attn_mask, dtype=np.float32).reshape(T, T)

    emT = np.ascontiguousarray(np.exp(mask).T)
    xT = [np.ascontiguousarray(x[b].T) for b in range(B)]
    scale = np.float32(1.0 / np.sqrt(DH))

    blocks = emT.reshape(NJ, 128, NI, 512)
    cls = np.full((NJ, NI), MIXED, dtype=np.int64)
    for jt in range(NJ):
        for ic in range(NI):
            sub = blocks[jt, :, ic, :]
            if not sub.any():
                cls[jt, ic] = SKIP
            elif np.all(sub == 1.0):
                cls[jt, ic] = NOMULT
    cls_key = tuple(cls.flatten().tolist())

    in_maps = []
    for c in range(8):
        b, g = c // 4, c % 4
        rows = slice(E * g, E * (g + 1))
        in_maps.append({
            "xt": xT[b],
            "wq": np.ascontiguousarray((Wq[rows, :] * scale).T),
            "wk": np.ascontiguousarray(Wk[rows, :].T),
            "wv": np.ascontiguousarray(Wv[rows, :].T),
            "wo": np.ascontiguousarray(Wo[:, rows].T),
            "em": emT,
            "onk": np.ones((128, 1), dtype=np.float32),
            "onp": np.ones((1, 128), dtype=np.float32),
        })

    global _LAST_IN_MAPS, _LAST_NC
    _LAST_IN_MAPS = in_maps
    nc = _get_nc(cls_key)
    _LAST_NC = nc
    res = run_bass_kernel_spmd(nc, in_maps, list(range(8)))
    outs = [r["out"] for r in res.results]
    full = np.stack([
        outs[0] + outs[1] + outs[2] + outs[3],
        outs[4] + outs[5] + outs[6] + outs[7],
    ]).astype(np.float32)
    return full
